# revision 18
# baseline (speedup 1.0000x reference)
"""EvolutionaryClusterVQ — Trainium2 Bass kernel (8-core data-parallel).

kernel(**inputs) takes the FULL inputs
    inputs  [16, 4096, 128] f32
    w_shape [1024, 64] f32
    w_color [16, 64] f32
and returns (quantized [16,4096,128] f32, vq_loss () f32,
             s_idx [16,4096] i32, c_idx [16,4096] i32).

Strategy (per core, tokens sharded 8 ways):
- codebooks are L2-normalized on device, scaled by 4096 and split into an
  fp16 hi+lo pair; same split for the (scaled) token vectors
- cosine argmax == argmax_k z . w_hat_k  (z-normalization and the snapping
  gain are monotone, so they are skipped); the dot products are computed on
  the PE with two K=128 fp16 matmuls per 512-code chunk:
      [zh;zl] x [wh;wl]  (hi*hi + lo*lo)   and   [zh;zl] x [wl;wh]  (cross)
  which together give z.w to ~fp32 accuracy at full bf16-class PE speed
- argmax in a single DVE pass with a custom fused op:
      cand[k] = (sim[k] >= running_max) ? k : -FLT_MAX ; accum = max(cand)
- codebook rows gathered by indirect DMA straight from the DRAM inputs
- vq_loss partials accumulated on ScalarE (square + accumulate), reduced
  on host; straight-through output equals the gathered rows.
"""

import numpy as np

import concourse.bass as bass
import concourse.bacc as bacc
import concourse.mybir as mybir
from concourse import tile
from concourse.bass_utils import run_bass_kernel_spmd
from concourse.masks import make_identity
from concourse.dve_ops import DveOp, OPS, _SUB_OPCODE_FOR_NAME, CUSTOM_DVE_SPECS
from concourse.dve_spec import Spec, Src0, Idx, MaxNeg, maxx, select, scan, AluOp, lower
from concourse.dve_uop import DveOpSpec

dt = mybir.dt
F32, F16, U32 = dt.float32, dt.float16, dt.uint32
AF = mybir.ActivationFunctionType
ALU = mybir.AluOpType
SCALE = 4096.0
N_CORES = 8
B, K, D = 16, 4096, 128
N_TOK_FULL = B * K
N_TOK = N_TOK_FULL // N_CORES       # 8192 per core
T = N_TOK // 128                    # 64 tiles
GRP = 8


# ---------------------------------------------------------------- custom op
def _ref_argmax_scan(in0, in1, c0, c1, c2):
    x = in0.reshape(in0.shape[0], -1).astype(np.float32)
    rm = np.maximum.accumulate(x, axis=1)
    idx = np.arange(x.shape[1], dtype=np.float32)[None, :]
    cand = np.where(x >= rm, idx, -np.finfo(np.float32).max)
    return cand.reshape(in0.shape), cand.max(axis=1, keepdims=True)


def _register_argmax_op() -> DveOp:
    name = "ARGMAX_SCAN_ANT"
    for op in OPS:
        if op.name == name:
            return op
    spec = Spec(
        body=select(Src0 >= scan(AluOp.MAX, Src0), Idx, MaxNeg),
        accum=maxx,
        reference=_ref_argmax_scan,
    )
    opcode = max(_SUB_OPCODE_FOR_NAME.values()) + 1
    assert opcode < 0x20
    _SUB_OPCODE_FOR_NAME[name] = opcode
    shas = {}
    for ver in ("v3",):
        s = DveOpSpec(name=name, opcode=opcode, uops=lower(spec, ver=ver),
                      rd1_en=False)
        shas[ver] = s.sha(ver)
    op = DveOp(name, spec, subdim=False, uops_sha=shas)
    OPS.append(op)
    CUSTOM_DVE_SPECS[name] = spec
    return op


ARGMAX_SCAN_ANT = _register_argmax_op()


def _ref_sqdiff(in0, in1, c0, c1, c2):
    a = in0.reshape(in0.shape[0], -1).astype(np.float32)
    b = in1.reshape(in1.shape[0], -1).astype(np.float32)
    d = (a - b) ** 2
    return d.reshape(in0.shape), d.sum(axis=1, keepdims=True)


def _register_sqdiff_op() -> DveOp:
    name = "SQDIFF_ACC_ANT"
    for op in OPS:
        if op.name == name:
            return op
    from concourse.dve_spec import Src1, sq
    spec = Spec(body=sq(Src0 - Src1), accum=AluOp.ADD, reference=_ref_sqdiff)
    opcode = max(_SUB_OPCODE_FOR_NAME.values()) + 1
    assert opcode < 0x20
    _SUB_OPCODE_FOR_NAME[name] = opcode
    shas = {}
    for ver in ("v3",):
        s = DveOpSpec(name=name, opcode=opcode, uops=lower(spec, ver=ver),
                      rd1_en=True)
        shas[ver] = s.sha(ver)
    op = DveOp(name, spec, subdim=False, uops_sha=shas)
    OPS.append(op)
    CUSTOM_DVE_SPECS[name] = spec
    return op


SQDIFF_ACC_ANT = _register_sqdiff_op()


# ---------------------------------------------------------------- builder
def build(nc, n_tok: int, grp: int = GRP, reps: int = 1, skip=()):
    """skip: subset of {"gather","gpsimd","argmax","loss","matmul"} for
    timing ablations (results are wrong when non-empty)."""
    T_ = n_tok // 128
    G = (T_ + grp - 1) // grp
    z_d = nc.dram_tensor("z", [n_tok, 128], F32, kind="ExternalInput")
    ws_d = nc.dram_tensor("ws", [1024, 64], F32, kind="ExternalInput")
    wc_d = nc.dram_tensor("wc", [16, 64], F32, kind="ExternalInput")
    q_d = nc.dram_tensor("q", [n_tok, 128], F32, kind="ExternalOutput")
    sidx_d = nc.dram_tensor("sidx", [128, T_], U32, kind="ExternalOutput")
    cidx_d = nc.dram_tensor("cidx", [128, T_], U32, kind="ExternalOutput")
    lossp_d = nc.dram_tensor("lossp", [128, 1], F32, kind="ExternalOutput")

    from contextlib import ExitStack
    with tile.TileContext(nc) as tc, ExitStack() as stk:
        consts = stk.enter_context(tc.tile_pool(name="consts", bufs=1))
        ident = consts.tile([128, 128], F16)
        make_identity(nc, ident)

        # stacks: [hiT; loT] and [loT; hiT] of the normalized, 4096-scaled
        # codebooks, fp16, shape [128, n_codes]
        stk1s = consts.tile([128, 1024], F16, tag="stk")
        stk2s = consts.tile([128, 1024], F16, tag="stk2")
        stk1c = consts.tile([128, 16], F16, tag="stkc")
        stk2c = consts.tile([128, 16], F16, tag="stkc2")

        with (
            tc.tile_pool(name="wprep", bufs=1) as wp,
            tc.tile_pool(name="wprep_ps", bufs=2, space="PSUM") as wps,
        ):
            wsb = wp.tile([128, 8, 64], F32)
            nc.sync.dma_start(wsb[:], ws_d.ap().rearrange("(t p) d -> p t d", p=128))
            wcb = wp.tile([16, 64], F32)
            nc.sync.dma_start(wcb[:], wc_d.ap())

            def prep(wtile, P_, stk1, stk2, nt):
                ssq = wp.tile([P_, nt], F32, tag="ssq")
                wn = wp.tile([P_, nt, 64], F32, tag="wn")
                hi = wp.tile([P_, nt, 64], F16, tag="hi")
                lo = wp.tile([P_, nt, 64], F16, tag="lo")
                for t in range(nt):
                    nc.scalar.activation(wn[:, t, :], wtile[:, t, :], AF.Square,
                                         accum_out=ssq[:, t : t + 1])
                nc.scalar.activation(ssq[:], ssq[:], AF.Sqrt)
                nc.vector.tensor_scalar_max(ssq[:], ssq[:], 1e-12)
                inv = wp.tile([P_, nt], F32, tag="inv")
                nc.vector.reciprocal(inv[:], ssq[:])
                nc.vector.tensor_scalar_mul(inv[:], inv[:], SCALE)
                for t in range(nt):
                    nc.scalar.activation(wn[:, t, :], wtile[:, t, :], AF.Copy,
                                         scale=inv[:, t : t + 1])
                nc.vector.tensor_copy(hi[:], wn[:])
                nc.vector.tensor_tensor(lo[:], wn[:], hi[:], op=ALU.subtract)
                for t in range(nt):
                    pst = wps.tile([64, 128], F16, tag="wpst")
                    col = slice(t * P_, (t + 1) * P_)
                    nc.tensor.transpose(pst[:, :P_], hi[:, t, :], ident[:P_, :P_])
                    nc.vector.tensor_copy(stk1[0:64, col], pst[:, :P_])
                    nc.vector.tensor_copy(stk2[64:128, col], pst[:, :P_])
                    pst2 = wps.tile([64, 128], F16, tag="wpst")
                    nc.tensor.transpose(pst2[:, :P_], lo[:, t, :], ident[:P_, :P_])
                    nc.vector.tensor_copy(stk1[64:128, col], pst2[:, :P_])
                    nc.vector.tensor_copy(stk2[0:64, col], pst2[:, :P_])

            prep(wsb, 128, stk1s, stk2s, 8)
            prep(wcb.rearrange("p (o d) -> p o d", o=1), 16, stk1c, stk2c, 1)

        sidxf = consts.tile([128, T_], F32, tag="sidxf")
        cidxf = consts.tile([128, T_], F32, tag="cidxf")
        sidx_u = consts.tile([128, T_], U32, tag="sidxu")
        cidx_u = consts.tile([128, T_], U32, tag="cidxu")
        lossbuf = consts.tile([128, 2 * G], F32, tag="lossbuf")
        # z resident for the whole shard; gathered rows land beside it
        z_all = consts.tile([128, T_, 128], F32, tag="zall")
        qs_all = consts.tile([128, T_, 64], F32, tag="qsall")
        qc_all = consts.tile([128, T_, 64], F32, tag="qcall")
        # dma_gather index layout: token i's index at [i % 16, i // 16]
        # (only partitions 0:16 are read; the rest must be in-bounds)
        I16 = dt.int16
        idx16sc = consts.tile([128, 2, T_ * 8], I16, tag="idx16sc")
        nc.vector.memset(idx16sc[:], 0)
        scidx16 = consts.tile([128, 2, T_], I16, tag="scidx16")

        PH = 32 if T_ >= 32 else T_          # tiles per phase
        NPH = (T_ + PH - 1) // PH
        with (
            tc.tile_pool(name="hl", bufs=2) as hlp,
            tc.tile_pool(name="zx", bufs=3) as zxp,
            tc.tile_pool(name="scr", bufs=2) as scrp,
            tc.tile_pool(name="csim", bufs=2) as csp,
            tc.tile_pool(name="psT", bufs=2, space="PSUM") as psT,
            tc.tile_pool(name="psS", bufs=2, space="PSUM") as psS,
            tc.tile_pool(name="psC", bufs=2, space="PSUM") as psC,
        ):
            if "argmax" in skip:
                nc.vector.memset(sidxf[:], 0.0)
                nc.vector.memset(cidxf[:], 0.0)

            def emit_loss(ph):
                p0, p1 = ph * PH, min((ph + 1) * PH, T_)
                for g in range(p0 // grp, (p1 + grp - 1) // grp):
                    t0, t1 = g * grp, min((g + 1) * grp, T_)
                    diff = scrp.tile([128, (t1 - t0) * 64], F32, tag="diff")
                    nc.vector._custom_dve(
                        SQDIFF_ACC_ANT, out=diff[:],
                        in0=qs_all[:, t0:t1, :],
                        in1=z_all[:, t0:t1, 0:64],
                        accum_out=lossbuf[:, 2 * g : 2 * g + 1])
                    diff2 = scrp.tile([128, (t1 - t0) * 64], F32, tag="diff")
                    nc.vector._custom_dve(
                        SQDIFF_ACC_ANT, out=diff2[:],
                        in0=qc_all[:, t0:t1, :],
                        in1=z_all[:, t0:t1, 64:128],
                        accum_out=lossbuf[:, 2 * g + 1 : 2 * g + 2])

            for _rep in range(reps):
              # prefetch the whole shard up front
              for g in range(G):
                  t0, t1 = g * grp, min((g + 1) * grp, T_)
                  nc.sync.dma_start(
                      z_all[:, t0:t1, :],
                      z_d.ap()[t0 * 128 : t1 * 128, :].rearrange(
                          "(t p) d -> p t d", p=128),
                  )
              for ph in range(NPH):
                p0, p1 = ph * PH, min((ph + 1) * PH, T_)
                pC = psC.tile([128, (p1 - p0) * 16], F32, tag="pC")
                for g in range(p0 // grp, (p1 + grp - 1) // grp):
                    t0, t1 = g * grp, min((g + 1) * grp, T_)
                    nt = t1 - t0
                    zg = z_all[:, t0:t1, :]
                    hls = hlp.tile([128, nt, 128], F16, tag="hls")
                    hlc = hlp.tile([128, nt, 128], F16, tag="hlc")
                    if "gpsimd" not in skip:
                        # hi = fp16(4096*z) on ScalarE; lo on VectorE
                        nc.scalar.activation(hls[:, :, 0:64], zg[:, :, 0:64],
                                             AF.Copy, scale=SCALE)
                        nc.scalar.activation(hlc[:, :, 0:64], zg[:, :, 64:128],
                                             AF.Copy, scale=SCALE)
                        nc.vector.scalar_tensor_tensor(
                            hls[:, :, 64:128], zg[:, :, 0:64], SCALE,
                            hls[:, :, 0:64], op0=ALU.mult, op1=ALU.subtract)
                        nc.vector.scalar_tensor_tensor(
                            hlc[:, :, 64:128], zg[:, :, 64:128], SCALE,
                            hlc[:, :, 0:64], op0=ALU.mult, op1=ALU.subtract)
                    for t in range(t0, t1):
                        ti = t - t0
                        pT = psT.tile([128, 256], F16, tag="pT")
                        nc.tensor.transpose(pT[:, 0:128], hls[:, ti, :], ident[:])
                        nc.tensor.transpose(pT[:, 128:256], hlc[:, ti, :],
                                            ident[:])
                        zxT = zxp.tile([128, 256], F16, tag="zxT")
                        nc.scalar.copy(zxT[:], pT[:])

                        pS = psS.tile([128, 1024], F32, tag="pS")
                        cslice = slice((t - p0) * 16, (t - p0 + 1) * 16)
                        if "matmul" not in skip:
                            for half in (0, 1):
                                cs = slice(half * 512, (half + 1) * 512)
                                nc.tensor.matmul(pS[:, cs], zxT[:, 0:128],
                                                 stk1s[:, cs], start=True,
                                                 stop=False)
                                nc.tensor.matmul(pS[:, cs], zxT[:, 0:128],
                                                 stk2s[:, cs], start=False,
                                                 stop=True)
                            nc.tensor.matmul(pC[:, cslice], zxT[:, 128:256],
                                             stk1c[:], start=True, stop=False)
                            nc.tensor.matmul(pC[:, cslice], zxT[:, 128:256],
                                             stk2c[:], start=False, stop=True)

                        if "argmax" not in skip:
                            scr = scrp.tile([128, 1024], F32, tag="scr")
                            nc.vector._custom_dve(ARGMAX_SCAN_ANT, out=scr[:],
                                                  in0=pS[:],
                                                  accum_out=sidxf[:, t : t + 1])
                # batched color argmax off one SBUF copy of the phase's sims
                if "argmax" not in skip:
                    csims = csp.tile([128, (p1 - p0) * 16], F32, tag="csims")
                    nc.scalar.copy(csims[:], pC[:])
                    for t in range(p0, p1):
                        scrc = scrp.tile([128, 16], F32, tag="scrc")
                        j = t - p0
                        nc.vector._custom_dve(
                            ARGMAX_SCAN_ANT, out=scrc[:],
                            in0=csims[:, j * 16 : (j + 1) * 16],
                            accum_out=cidxf[:, t : t + 1])

                # ---- per-phase gather + loss + q writeback ----
                nph = p1 - p0
                nc.vector.tensor_copy(scidx16[:, 0, p0:p1], sidxf[:, p0:p1])
                nc.vector.tensor_copy(scidx16[:, 1, p0:p1], cidxf[:, p0:p1])
                if "gather" not in skip:
                    # idx16[r, c, 8t+k] = scidx16[16k+r, c, t]
                    for k in range(8):
                        eng = nc.sync if (k % 2 == 0) else nc.scalar
                        eng.dma_start(
                            idx16sc[0:16, 0, p0 * 8 + k : p1 * 8 : 8],
                            scidx16[16 * k : 16 * (k + 1), 0, p0:p1])
                        eng.dma_start(
                            idx16sc[0:16, 1, p0 * 8 + k : p1 * 8 : 8],
                            scidx16[16 * k : 16 * (k + 1), 1, p0:p1])
                    # HW SWDGE reads idxs from all 8 16-partition blocks
                    # (one per Q7 core) — replicate by doubling
                    cols = slice(p0 * 8, p1 * 8)
                    nc.sync.dma_start(idx16sc[16:32, :, cols],
                                      idx16sc[0:16, :, cols])
                    nc.scalar.dma_start(idx16sc[32:64, :, cols],
                                        idx16sc[0:32, :, cols])
                    nc.sync.dma_start(idx16sc[64:128, :, cols],
                                      idx16sc[0:64, :, cols])
                    # SWDGE descriptor ring limits one gather to ~1024 idxs
                    for b0 in range(p0, p1, 8):
                        b1 = min(b0 + 8, p1)
                        nb = (b1 - b0) * 128
                        nc.gpsimd.dma_gather(
                            out_ap=qs_all[:, b0:b1, :], in_ap=ws_d.ap(),
                            idxs_ap=idx16sc[:, 0, b0 * 8 : b1 * 8],
                            num_idxs=nb, num_idxs_reg=nb, elem_size=64)
                        nc.gpsimd.dma_gather(
                            out_ap=qc_all[:, b0:b1, :], in_ap=wc_d.ap(),
                            idxs_ap=idx16sc[:, 1, b0 * 8 : b1 * 8],
                            num_idxs=nb, num_idxs_reg=nb, elem_size=64)
                if ph > 0:
                    if "loss" not in skip:
                        emit_loss(ph - 1)  # previous phase's gather is done
                    q0, q1 = (ph - 1) * PH, p0
                    nc.sync.dma_start(
                        q_d.ap().rearrange("(t p) d -> p t d", p=128)[:, q0:q1,
                                                                      0:64],
                        qs_all[:, q0:q1, :])
                    nc.sync.dma_start(
                        q_d.ap().rearrange("(t p) d -> p t d", p=128)[:, q0:q1,
                                                                      64:128],
                        qc_all[:, q0:q1, :])
              if "loss" not in skip:
                  emit_loss(NPH - 1)
              q0, q1 = (NPH - 1) * PH, T_
              nc.sync.dma_start(
                  q_d.ap().rearrange("(t p) d -> p t d", p=128)[:, q0:q1, 0:64],
                  qs_all[:, q0:q1, :])
              nc.sync.dma_start(
                  q_d.ap().rearrange("(t p) d -> p t d", p=128)[:, q0:q1, 64:128],
                  qc_all[:, q0:q1, :])

            lossp = consts.tile([128, 1], F32, tag="lossp")
            nc.vector.reduce_sum(lossp[:], lossbuf[:], axis=mybir.AxisListType.X)
            nc.vector.tensor_copy(sidx_u[:], sidxf[:])
            nc.vector.tensor_copy(cidx_u[:], cidxf[:])
            nc.sync.dma_start(lossp_d.ap(), lossp[:])
            nc.sync.dma_start(sidx_d.ap(), sidx_u[:])
            nc.sync.dma_start(cidx_d.ap(), cidx_u[:])
    return nc


_COMPILED = {}


def _get_compiled():
    if "nc" not in _COMPILED:
        nc = bacc.Bacc("TRN2", target_bir_lowering=False, debug=False,
                       num_devices=N_CORES, dynamic_dma_scratch_size=49152)
        build(nc, N_TOK, GRP)
        nc.compile()
        _COMPILED["nc"] = nc
    return _COMPILED["nc"]


def kernel(inputs, w_shape, w_color, _trace=False, _results=None):
    inputs = np.ascontiguousarray(inputs, dtype=np.float32)
    w_shape = np.ascontiguousarray(w_shape, dtype=np.float32)
    w_color = np.ascontiguousarray(w_color, dtype=np.float32)
    flat = inputs.reshape(N_TOK_FULL, D)

    nc = _get_compiled()
    in_maps = [
        {"z": np.ascontiguousarray(flat[c * N_TOK : (c + 1) * N_TOK]),
         "ws": w_shape, "wc": w_color}
        for c in range(N_CORES)
    ]
    res = run_bass_kernel_spmd(nc, in_maps, core_ids=list(range(N_CORES)),
                               trace=_trace)
    if _results is not None:
        _results.append(res)

    q = np.concatenate([r["q"] for r in res.results], axis=0)
    sidx = np.concatenate(
        [r["sidx"].T.reshape(-1) for r in res.results]).astype(np.int32)
    cidx = np.concatenate(
        [r["cidx"].T.reshape(-1) for r in res.results]).astype(np.int32)
    loss_sum = np.sum([r["lossp"].sum(dtype=np.float64) for r in res.results])
    vq_loss = np.float32(loss_sum * 0.25 / (N_TOK_FULL * D))

    return (q.reshape(B, K, D), vq_loss,
            sidx.reshape(B, K), cidx.reshape(B, K))
